# revision 3
# baseline (speedup 1.0000x reference)
"""CTC beam search (topk_masking) for Trainium2, 8 NeuronCores.

Time-sharded: core c owns frames [c*256, (c+1)*256). Per 128-partition tile
(8 frames x 16 rows of 3144):
  - DVE `max`:        top-8 values per row (exact, descending)    [1 pass]
  - DVE `max_index`:  their indices within the row                [1 pass]
  - ScalarE Exp (no shift, randn-scale data) with per-row accum   [1 pass]
Optionally the first G tiles instead use the gpsimd `topk` custom
instruction (exact top-256 per frame, values+indices) to offload the DVE.

Host: merge row candidates (top-8x16 = 128/frame), provably-sound repair of
rows that may hide >8 top-32 members (threshold test vs the 32nd-best
candidate), per-frame log-sum-exp, then the tiny sequential fp32 beam
recurrence with jax-compatible lowest-flat-index tie-breaking.
"""

import numpy as np

T, V = 2048, 50257
NCORES = 8
FPC = T // NCORES            # 256 frames per core
VP = 50304                   # padded vocab (gpsimd topk: %128==0, >50000)
W = VP // 16                 # 3144 per partition row
TILES = FPC // 8             # 32 tiles of 8 frames each
K = 256                      # gpsimd topk k
PAD = np.float32(-1e4)       # below any real logit; exp underflows to 0
NSEL = 64                    # candidates per frame fed to the recurrence
G_TILES = 0                  # leading tiles per core handled by gpsimd topk

_CACHE = {}


def _emit_topk(nc, out_ap, in_ap, tokens, vocab_size, k):
    """InstTopk on gpsimd accepting symbolic tile APs (same lowering as
    BassGpSimd.topk, minus its concrete-SBTensorHandle assert)."""
    import concourse.bass_isa as bass_isa

    g = nc.gpsimd
    return g.add_instruction(
        bass_isa.InstTopk(
            name=f"I-{nc.next_id()}",
            ins=[g.lower_ap(in_ap, for_isa=True)],
            outs=[g.lower_ap(out_ap, for_isa=True)],
            _tokens=tokens,
            _n=vocab_size,
            _k=k,
        )
    )


def _build_nc():
    import concourse.mybir as mybir
    from concourse.bacc import Bacc
    from concourse.tile import TileContext

    F32, U32 = mybir.dt.float32, mybir.dt.uint32
    nc = Bacc()
    enc = nc.dram_tensor("enc", [FPC * 16, W], F32, kind="ExternalInput")
    mxo = nc.dram_tensor("mx", [FPC * 16, 8], F32, kind="ExternalOutput")
    mio = nc.dram_tensor("mi", [FPC * 16, 8], U32, kind="ExternalOutput")
    zo = nc.dram_tensor("z", [FPC * 16, 1], F32, kind="ExternalOutput")
    tko = nc.dram_tensor(
        "tk", [max(G_TILES, 1) * 128, 2 * (K // 16)], U32, kind="ExternalOutput"
    )

    with TileContext(nc) as tc:
        with (
            tc.tile_pool(name="pin", bufs=4) as pin,
            tc.tile_pool(name="pout", bufs=6) as pout,
            tc.tile_pool(name="pscr", bufs=1) as pscr,
        ):
            scr = pscr.tile([128, W], F32)  # exp stream output, never read
            for i in range(TILES):
                rows = slice(i * 128, (i + 1) * 128)
                t = pin.tile([128, W], F32)
                nc.sync.dma_start(out=t[:], in_=enc[rows, :])
                if i < G_TILES:
                    o = pout.tile([128, 2 * (K // 16)], U32, tag="tk")
                    _emit_topk(nc, o[:], t[:], tokens=8, vocab_size=VP, k=K)
                    nc.sync.dma_start(
                        out=tko[i * 128 : (i + 1) * 128, :], in_=o[:]
                    )
                else:
                    mx = pout.tile([128, 8], F32, tag="mx")
                    mi = pout.tile([128, 8], U32, tag="mi")
                    nc.vector.max(out=mx[:], in_=t[:])
                    nc.vector.max_index(out=mi[:], in_max=mx[:], in_values=t[:])
                    nc.sync.dma_start(out=mxo[rows, :], in_=mx[:])
                    nc.sync.dma_start(out=mio[rows, :], in_=mi[:])
                z = pout.tile([128, 1], F32, tag="z")
                nc.scalar.activation(
                    scr[:],
                    t[:],
                    mybir.ActivationFunctionType.Exp,
                    bias=0.0,
                    scale=1.0,
                    accum_out=z[:],
                )
                nc.sync.dma_start(out=zo[rows, :], in_=z[:])
    nc.finalize()
    return nc


def _get_nc():
    if "nc" not in _CACHE:
        _CACHE["nc"] = _build_nc()
    return _CACHE["nc"]


def _shard_inputs(enc_out):
    in_maps = []
    for c in range(NCORES):
        buf = np.full((FPC, VP), PAD, dtype=np.float32)
        buf[:, :V] = enc_out[c * FPC : (c + 1) * FPC]
        in_maps.append({"enc": np.ascontiguousarray(buf.reshape(FPC * 16, W))})
    return in_maps


def _run_device(enc_out, **kw):
    from concourse.bass_utils import run_bass_kernel_spmd

    nc = _get_nc()
    res = run_bass_kernel_spmd(
        nc, _shard_inputs(enc_out), core_ids=list(range(NCORES)), **kw
    )
    mx = np.concatenate([r["mx"].reshape(FPC, 16, 8) for r in res.results])
    mi = np.concatenate(
        [r["mi"].reshape(FPC, 16, 8).astype(np.int64) for r in res.results]
    )
    zrow = np.concatenate([r["z"].reshape(FPC, 16) for r in res.results])
    if G_TILES:
        tk = np.stack([r["tk"] for r in res.results])  # [8, G*128, 32]
    else:
        tk = None
    return mx, mi, zrow, tk, res


def _candidates(enc_out, mx, mi, zrow, tk):
    """Per-frame NSEL candidates (value desc, index-asc tie-break), exact."""
    gf = G_TILES * 8  # leading frames per core handled by gpsimd topk
    vals = mx  # [T, 16, 8] exact f32 row top-8, descending
    gidx = mi + (np.arange(16, dtype=np.int64) * W).reshape(1, 16, 1)

    cand_v = vals.reshape(T, 128)
    cand_i = gidx.reshape(T, 128)

    # Frame max for log-softmax shift (bitwise: copies of raw values).
    m = cand_v.max(axis=1)

    # threshold = 32nd best captured candidate per frame
    tau = -np.partition(-cand_v, 31, axis=1)[:, 31]
    # rows that might hide additional >= tau elements beyond their top-8,
    # plus rows whose index list shows duplicates/unmatched (-1) entries
    row8 = vals[:, :, 7]
    suspicious = row8 >= tau[:, None]
    dup = np.zeros((T, 16), dtype=bool)
    si = np.sort(mi, axis=2)
    dup |= (si[:, :, 1:] == si[:, :, :-1]).any(axis=2)
    dup |= (mi < 0).any(axis=2) | (mi >= W).any(axis=2)
    repair = suspicious | dup

    sel_v = np.empty((T, NSEL), np.float32)
    sel_i = np.empty((T, NSEL), np.int64)

    # fast path: top-NSEL by (value desc, index asc) out of 128
    order = np.lexsort((cand_i, -cand_v), axis=1)[:, :NSEL]
    sel_v[:] = np.take_along_axis(cand_v, order, axis=1)
    sel_i[:] = np.take_along_axis(cand_i, order, axis=1)

    # exact repair for rare frames
    for t in np.nonzero(repair.any(axis=1))[0]:
        fv, fi = [], []
        for r in range(16):
            lo, hi = r * W, min((r + 1) * W, V)
            if repair[t, r]:
                row = enc_out[t, lo:hi]
                keep = np.nonzero(row >= min(tau[t], vals[t, r, 7]))[0]
                fv.append(row[keep])
                fi.append(keep + lo)
            else:
                fv.append(vals[t, r])
                fi.append(gidx[t, r])
        fv = np.concatenate(fv)
        fi = np.concatenate(fi)
        o = np.lexsort((fi, -fv))[:NSEL]
        sel_v[t, : len(o)] = fv[o]
        sel_i[t, : len(o)] = fi[o]

    # gpsimd-topk frames: exact sorted top-256 (ascending) + indices
    if tk is not None and gf:
        kk = K // 16
        for c in range(NCORES):
            blk = tk[c].reshape(gf, 16, 2 * kk)
            v = (
                np.ascontiguousarray(blk[:, :, :kk])
                .reshape(gf, K)
                .view(np.float32)
            )
            ix = blk[:, :, kk:].reshape(gf, K).astype(np.int64)
            fr = slice(c * FPC, c * FPC + gf)
            # ascending; take last NSEL, then order (value desc, idx asc)
            vv = v[:, -NSEL:]
            ii = ix[:, -NSEL:]
            o = np.lexsort((ii, -vv), axis=1)
            sel_v[fr] = np.take_along_axis(vv, o, axis=1)
            sel_i[fr] = np.take_along_axis(ii, o, axis=1)
            m[fr] = v[:, -1]

    return sel_v, sel_i, m


def _host_finish(enc_out, mx, mi, zrow, tk, beam_width):
    sel_v, sel_i, m = _candidates(enc_out, mx, mi, zrow, tk)

    # log-sum-exp: device rows give sum(exp(x)) per row (no shift; exp of
    # the -1e4 pad underflows to exactly 0)
    z64 = zrow.astype(np.float64).sum(axis=1)
    lse = (np.log(z64) - m.astype(np.float64)).astype(np.float32)  # [T]

    logp = (sel_v - m[:, None]).astype(np.float32)
    logp = (logp - lse[:, None]).astype(np.float32)

    bw = int(beam_width)
    scores = np.full((bw,), np.float32(-1e30), dtype=np.float32)
    scores[0] = np.float32(0.0)
    toks = np.empty((T, bw), np.int32)
    hyp = np.empty((T, bw), np.int32)
    fi_base = np.arange(bw, dtype=np.int64)[:, None] * V
    for t in range(T):
        c = (scores[:, None] + logp[t][None, :]).astype(np.float32).ravel()
        fi = (fi_base + sel_i[t][None, :]).ravel()
        ordr = np.lexsort((fi, -c))[:bw]  # value desc, flat index asc
        scores = c[ordr]
        fo = fi[ordr]
        toks[t] = (fo % V).astype(np.int32)
        hyp[t] = (fo // V).astype(np.int32)
    return scores, toks, hyp


def kernel(enc_out, beam_width):
    enc_out = np.asarray(enc_out, dtype=np.float32)
    assert enc_out.shape == (T, V), enc_out.shape
    mx, mi, zrow, tk, _ = _run_device(enc_out)
    return _host_finish(enc_out, mx, mi, zrow, tk, beam_width)


# revision 5
# speedup vs baseline: 2.0948x; 2.0948x over previous
"""CTC beam search (topk_masking) for Trainium2, 8 NeuronCores.

Time-sharded: core c owns frames [c*256, (c+1)*256). Per 128-partition tile
(8 frames x 16 rows of 3144):
  - DVE `max`:        top-8 values per row (exact, descending)    [1 pass]
  - DVE `max_index`:  their indices within the row                [1 pass]
  - ScalarE Exp (no shift, randn-scale data) with per-row accum   [1 pass]
Optionally the first G tiles instead use the gpsimd `topk` custom
instruction (exact top-256 per frame, values+indices) to offload the DVE.

Host: merge row candidates (top-8x16 = 128/frame), provably-sound repair of
rows that may hide >8 top-32 members (threshold test vs the 32nd-best
candidate), per-frame log-sum-exp, then the tiny sequential fp32 beam
recurrence with jax-compatible lowest-flat-index tie-breaking.
"""

import numpy as np

T, V = 2048, 50257
NCORES = 8
FPC = T // NCORES            # 256 frames per core
VP = 50304                   # padded vocab (gpsimd topk: %128==0, >50000)
W = VP // 16                 # 3144 per partition row
TILES = FPC // 8             # 32 tiles of 8 frames each
K = 256                      # gpsimd topk k
PAD = np.float32(-1e4)       # below any real logit; exp underflows to 0
NSEL = 64                    # candidates per frame fed to the recurrence
G_TILES = 0                  # leading tiles per core handled by gpsimd topk

_CACHE = {}


def _emit_topk(nc, out_ap, in_ap, tokens, vocab_size, k):
    """InstTopk on gpsimd accepting symbolic tile APs (same lowering as
    BassGpSimd.topk, minus its concrete-SBTensorHandle assert)."""
    import concourse.bass_isa as bass_isa

    g = nc.gpsimd
    return g.add_instruction(
        bass_isa.InstTopk(
            name=f"I-{nc.next_id()}",
            ins=[g.lower_ap(in_ap, for_isa=True)],
            outs=[g.lower_ap(out_ap, for_isa=True)],
            _tokens=tokens,
            _n=vocab_size,
            _k=k,
        )
    )


SPANS = 4                    # 3144-wide spans per partition line
LW = SPANS * W               # 12576 elements per partition line (50 KiB)
LINES = FPC * 16 // SPANS    # 1024 lines per core
STILES = LINES // 128        # 8 supertiles of [128, 12576]


def _build_nc():
    import concourse.mybir as mybir
    from concourse.bacc import Bacc
    from concourse.tile import TileContext

    F32, U32 = mybir.dt.float32, mybir.dt.uint32
    nc = Bacc()
    enc = nc.dram_tensor("enc", [LINES, LW], F32, kind="ExternalInput")
    mxo = nc.dram_tensor("mx", [LINES, SPANS * 8], F32, kind="ExternalOutput")
    mio = nc.dram_tensor("mi", [LINES, SPANS * 8], U32, kind="ExternalOutput")
    zo = nc.dram_tensor("z", [LINES, SPANS], F32, kind="ExternalOutput")

    with TileContext(nc) as tc:
        with (
            tc.tile_pool(name="pin", bufs=3) as pin,
            tc.tile_pool(name="pout", bufs=6) as pout,
            tc.tile_pool(name="pscr", bufs=1) as pscr,
        ):
            scr = pscr.tile([128, W], F32)  # exp stream output, never read
            for i in range(STILES):
                rows = slice(i * 128, (i + 1) * 128)
                t = pin.tile([128, LW], F32)
                half = LW // 2
                # split the load across both HWDGE queues (SP + ACT)
                nc.sync.dma_start(out=t[:, :half], in_=enc[rows, :half])
                nc.scalar.dma_start(out=t[:, half:], in_=enc[rows, half:])
                mx = pout.tile([128, SPANS * 8], F32, tag="mx")
                mi = pout.tile([128, SPANS * 8], U32, tag="mi")
                z = pout.tile([128, SPANS], F32, tag="z")
                for s in range(SPANS):
                    sl = slice(s * W, (s + 1) * W)
                    c8 = slice(s * 8, (s + 1) * 8)
                    nc.vector.max(out=mx[:, c8], in_=t[:, sl])
                    nc.vector.max_index(
                        out=mi[:, c8], in_max=mx[:, c8], in_values=t[:, sl]
                    )
                    nc.scalar.activation(
                        scr[:],
                        t[:, sl],
                        mybir.ActivationFunctionType.Exp,
                        bias=0.0,
                        scale=1.0,
                        accum_out=z[:, s : s + 1],
                    )
                nc.sync.dma_start(out=mxo[rows, :], in_=mx[:])
                nc.sync.dma_start(out=mio[rows, :], in_=mi[:])
                nc.sync.dma_start(out=zo[rows, :], in_=z[:])
    nc.finalize()
    return nc


def _get_nc():
    if "nc" not in _CACHE:
        _CACHE["nc"] = _build_nc()
    return _CACHE["nc"]


def _shard_inputs(enc_out):
    in_maps = []
    for c in range(NCORES):
        buf = np.full((FPC, VP), PAD, dtype=np.float32)
        buf[:, :V] = enc_out[c * FPC : (c + 1) * FPC]
        in_maps.append({"enc": np.ascontiguousarray(buf.reshape(LINES, LW))})
    return in_maps


def _run_device(enc_out, **kw):
    from concourse.bass_utils import run_bass_kernel_spmd

    nc = _get_nc()
    res = run_bass_kernel_spmd(
        nc, _shard_inputs(enc_out), core_ids=list(range(NCORES)), **kw
    )
    # [LINES, SPANS*8] lines (frame-quarters) -> [FPC, 16 spans, 8]
    mx = np.concatenate([r["mx"].reshape(FPC, 16, 8) for r in res.results])
    mi = np.concatenate(
        [r["mi"].reshape(FPC, 16, 8).astype(np.int64) for r in res.results]
    )
    zrow = np.concatenate([r["z"].reshape(FPC, 16) for r in res.results])
    return mx, mi, zrow, None, res


def _candidates(enc_out, mx, mi, zrow, tk):
    """Per-frame NSEL candidates (value desc, index-asc tie-break), exact."""
    gf = G_TILES * 8  # leading frames per core handled by gpsimd topk
    vals = mx  # [T, 16, 8] exact f32 row top-8, descending
    gidx = mi + (np.arange(16, dtype=np.int64) * W).reshape(1, 16, 1)

    cand_v = vals.reshape(T, 128)
    cand_i = gidx.reshape(T, 128)

    # Frame max for log-softmax shift (bitwise: copies of raw values).
    m = cand_v.max(axis=1)

    # threshold = 32nd best captured candidate per frame
    tau = -np.partition(-cand_v, 31, axis=1)[:, 31]
    # rows that might hide additional >= tau elements beyond their top-8,
    # plus rows whose index list shows duplicates/unmatched (-1) entries
    row8 = vals[:, :, 7]
    suspicious = row8 >= tau[:, None]
    dup = np.zeros((T, 16), dtype=bool)
    si = np.sort(mi, axis=2)
    dup |= (si[:, :, 1:] == si[:, :, :-1]).any(axis=2)
    dup |= (mi < 0).any(axis=2) | (mi >= W).any(axis=2)
    repair = suspicious | dup

    sel_v = np.empty((T, NSEL), np.float32)
    sel_i = np.empty((T, NSEL), np.int64)

    # fast path: top-NSEL by (value desc, index asc) out of 128
    order = np.lexsort((cand_i, -cand_v), axis=1)[:, :NSEL]
    sel_v[:] = np.take_along_axis(cand_v, order, axis=1)
    sel_i[:] = np.take_along_axis(cand_i, order, axis=1)

    # exact repair for rare frames
    for t in np.nonzero(repair.any(axis=1))[0]:
        fv, fi = [], []
        for r in range(16):
            lo, hi = r * W, min((r + 1) * W, V)
            if repair[t, r]:
                row = enc_out[t, lo:hi]
                keep = np.nonzero(row >= min(tau[t], vals[t, r, 7]))[0]
                fv.append(row[keep])
                fi.append(keep + lo)
            else:
                fv.append(vals[t, r])
                fi.append(gidx[t, r])
        fv = np.concatenate(fv)
        fi = np.concatenate(fi)
        o = np.lexsort((fi, -fv))[:NSEL]
        sel_v[t, : len(o)] = fv[o]
        sel_i[t, : len(o)] = fi[o]

    # gpsimd-topk frames: exact sorted top-256 (ascending) + indices
    if tk is not None and gf:
        kk = K // 16
        for c in range(NCORES):
            blk = tk[c].reshape(gf, 16, 2 * kk)
            v = (
                np.ascontiguousarray(blk[:, :, :kk])
                .reshape(gf, K)
                .view(np.float32)
            )
            ix = blk[:, :, kk:].reshape(gf, K).astype(np.int64)
            fr = slice(c * FPC, c * FPC + gf)
            # ascending; take last NSEL, then order (value desc, idx asc)
            vv = v[:, -NSEL:]
            ii = ix[:, -NSEL:]
            o = np.lexsort((ii, -vv), axis=1)
            sel_v[fr] = np.take_along_axis(vv, o, axis=1)
            sel_i[fr] = np.take_along_axis(ii, o, axis=1)
            m[fr] = v[:, -1]

    return sel_v, sel_i, m


def _host_finish(enc_out, mx, mi, zrow, tk, beam_width):
    sel_v, sel_i, m = _candidates(enc_out, mx, mi, zrow, tk)

    # log-sum-exp: device rows give sum(exp(x)) per row (no shift; exp of
    # the -1e4 pad underflows to exactly 0)
    z64 = zrow.astype(np.float64).sum(axis=1)
    lse = (np.log(z64) - m.astype(np.float64)).astype(np.float32)  # [T]

    logp = (sel_v - m[:, None]).astype(np.float32)
    logp = (logp - lse[:, None]).astype(np.float32)

    bw = int(beam_width)
    scores = np.full((bw,), np.float32(-1e30), dtype=np.float32)
    scores[0] = np.float32(0.0)
    toks = np.empty((T, bw), np.int32)
    hyp = np.empty((T, bw), np.int32)
    fi_base = np.arange(bw, dtype=np.int64)[:, None] * V
    for t in range(T):
        c = (scores[:, None] + logp[t][None, :]).astype(np.float32).ravel()
        fi = (fi_base + sel_i[t][None, :]).ravel()
        ordr = np.lexsort((fi, -c))[:bw]  # value desc, flat index asc
        scores = c[ordr]
        fo = fi[ordr]
        toks[t] = (fo % V).astype(np.int32)
        hyp[t] = (fo // V).astype(np.int32)
    return scores, toks, hyp


def kernel(enc_out, beam_width):
    enc_out = np.asarray(enc_out, dtype=np.float32)
    assert enc_out.shape == (T, V), enc_out.shape
    mx, mi, zrow, tk, _ = _run_device(enc_out)
    return _host_finish(enc_out, mx, mi, zrow, tk, beam_width)


# revision 6
# speedup vs baseline: 2.8002x; 1.3368x over previous
"""CTC beam search (topk_masking) for Trainium2, 8 NeuronCores.

Time-sharded: core c owns frames [c*256, (c+1)*256). Device computes, per
frame (50304-padded vocab = 1048 spans of 48):
  - DVE tensor_reduce(max): span maxima (one pass over all data)
  - ScalarE Exp (randn-scale data, no shift) accumulated per 3144-chunk
Host: the 32-of-1048 span-maxima threshold argument — every frame-top-32
element lives in a span whose max is >= the 64th-largest span max (subset
order statistics) — so gathering all elements >= that threshold from the
~64-90 flagged spans reconstructs exact top-64 values+indices; then
per-frame log-sum-exp and the tiny sequential fp32 beam recurrence with
jax-compatible lowest-flat-index tie-breaking.
"""

import numpy as np

T, V = 2048, 50257
NCORES = 8
FPC = T // NCORES            # 256 frames per core
VP = 50304                   # padded vocab
W = 3144                     # exp-accumulation chunk (16 per frame)
SW = 48                      # span width for the max pass
NSPANS = VP // SW            # 1048 spans per frame
PAD = np.float32(-1e4)       # below any real logit; exp underflows to 0
NSEL = 64                    # candidates per frame fed to the recurrence

SPANS = 4                    # 3144-chunks per partition line
LW = SPANS * W               # 12576 elements per line (50 KiB descriptors)
LINES = FPC * 16 // SPANS    # 1024 lines per core
STILES = LINES // 128        # 8 supertiles of [128, 12576]
SPL = LW // SW               # 262 spans per line

_CACHE = {}


def _build_nc():
    import concourse.mybir as mybir
    from concourse.bacc import Bacc
    from concourse.tile import TileContext

    F32 = mybir.dt.float32
    nc = Bacc()
    enc = nc.dram_tensor("enc", [LINES, LW], F32, kind="ExternalInput")
    smo = nc.dram_tensor("sm", [LINES, SPL], F32, kind="ExternalOutput")
    zo = nc.dram_tensor("z", [LINES, SPANS], F32, kind="ExternalOutput")

    with TileContext(nc) as tc:
        with (
            tc.tile_pool(name="pin", bufs=3) as pin,
            tc.tile_pool(name="pout", bufs=6) as pout,
            tc.tile_pool(name="pscr", bufs=1) as pscr,
        ):
            scr = pscr.tile([128, W], F32)  # exp stream output, never read
            for i in range(STILES):
                rows = slice(i * 128, (i + 1) * 128)
                t = pin.tile([128, LW], F32)
                half = LW // 2
                # split the load across both HWDGE queues (SP + ACT)
                nc.sync.dma_start(out=t[:, :half], in_=enc[rows, :half])
                nc.scalar.dma_start(out=t[:, half:], in_=enc[rows, half:])
                sm = pout.tile([128, SPL], F32, tag="sm")
                z = pout.tile([128, SPANS], F32, tag="z")
                nc.vector.reduce_max(
                    sm[:],
                    t[:].rearrange("p (s e) -> p s e", e=SW),
                    axis=mybir.AxisListType.X,
                )
                for s in range(SPANS):
                    nc.scalar.activation(
                        scr[:],
                        t[:, s * W : (s + 1) * W],
                        mybir.ActivationFunctionType.Exp,
                        bias=0.0,
                        scale=1.0,
                        accum_out=z[:, s : s + 1],
                    )
                nc.sync.dma_start(out=smo[rows, :], in_=sm[:])
                nc.sync.dma_start(out=zo[rows, :], in_=z[:])
    nc.finalize()
    return nc


def _get_nc():
    if "nc" not in _CACHE:
        _CACHE["nc"] = _build_nc()
    return _CACHE["nc"]


def _shard_inputs(enc_out):
    in_maps, pads = [], []
    for c in range(NCORES):
        buf = np.full((FPC, VP), PAD, dtype=np.float32)
        buf[:, :V] = enc_out[c * FPC : (c + 1) * FPC]
        pads.append(buf)
        in_maps.append({"enc": buf.reshape(LINES, LW)})
    return in_maps, pads


def _run_device(enc_out, **kw):
    from concourse.bass_utils import run_bass_kernel_spmd

    nc = _get_nc()
    in_maps, pads = _shard_inputs(enc_out)
    res = run_bass_kernel_spmd(
        nc, in_maps, core_ids=list(range(NCORES)), **kw
    )
    # [LINES, SPL] -> per-frame [FPC, 1048]; z -> [FPC, 16]
    sm = np.concatenate([r["sm"].reshape(FPC, NSPANS) for r in res.results])
    zrow = np.concatenate([r["z"].reshape(FPC, 16) for r in res.results])
    return sm, zrow, pads, res


def _candidates(sm, pads):
    """Exact per-frame top-NSEL (value desc, index asc) from span maxima."""
    m = sm.max(axis=1)  # frame max, bitwise exact
    # NSEL-th largest span max: sound gather threshold (subset order stats)
    tau = -np.partition(-sm, NSEL - 1, axis=1)[:, NSEL - 1]  # [T]

    sel_v = np.full((T, NSEL), np.float32(-np.inf), dtype=np.float32)
    sel_i = np.zeros((T, NSEL), np.int64)
    for c in range(NCORES):
        pv = pads[c].reshape(FPC, NSPANS, SW)
        fr = slice(c * FPC, (c + 1) * FPC)
        smc, tauc = sm[fr], tau[fr]
        fmask = smc >= tauc[:, None]  # [FPC, 1048]
        fidx, sidx = np.nonzero(fmask)
        blocks = pv[fidx, sidx]  # [n, 48]
        keep = blocks >= tauc[fidx][:, None]
        bi, off = np.nonzero(keep)
        vals = blocks[bi, off]
        gidx = sidx[bi] * SW + off
        frame = fidx[bi]
        # per-frame top-NSEL by (value desc, index asc)
        order = np.lexsort((gidx, -vals, frame))
        frame_o = frame[order]
        starts = np.searchsorted(frame_o, np.arange(FPC))
        ends = np.searchsorted(frame_o, np.arange(FPC), side="right")
        for f in range(FPC):
            s, e = starts[f], min(ends[f], starts[f] + NSEL)
            n = e - s
            sel_v[c * FPC + f, :n] = vals[order[s:e]]
            sel_i[c * FPC + f, :n] = gidx[order[s:e]]
    return sel_v, sel_i, m


def _host_finish(sm, zrow, pads, beam_width):
    sel_v, sel_i, m = _candidates(sm, pads)

    # log-sum-exp from device row sums of exp(x) (pad rows add exactly 0)
    z64 = zrow.astype(np.float64).sum(axis=1)
    lse = (np.log(z64) - m.astype(np.float64)).astype(np.float32)  # [T]

    logp = (sel_v - m[:, None]).astype(np.float32)
    logp = (logp - lse[:, None]).astype(np.float32)

    bw = int(beam_width)
    scores = np.full((bw,), np.float32(-1e30), dtype=np.float32)
    scores[0] = np.float32(0.0)
    toks = np.empty((T, bw), np.int32)
    hyp = np.empty((T, bw), np.int32)
    fi_base = np.arange(bw, dtype=np.int64)[:, None] * V
    for t in range(T):
        c = (scores[:, None] + logp[t][None, :]).astype(np.float32).ravel()
        fi = (fi_base + sel_i[t][None, :]).ravel()
        ordr = np.lexsort((fi, -c))[:bw]  # value desc, flat index asc
        scores = c[ordr]
        fo = fi[ordr]
        toks[t] = (fo % V).astype(np.int32)
        hyp[t] = (fo // V).astype(np.int32)
    return scores, toks, hyp


def kernel(enc_out, beam_width):
    enc_out = np.asarray(enc_out, dtype=np.float32)
    assert enc_out.shape == (T, V), enc_out.shape
    sm, zrow, pads, _ = _run_device(enc_out)
    return _host_finish(sm, zrow, pads, beam_width)
